# revision 1
# baseline (speedup 1.0000x reference)
"""BitLinear kernel for Trainium2, 8 NeuronCores, column-parallel.

y[t, o] = sum_i x[t, i] * sign(W[o, i]) * scale[o]
  x: [8192, 4096] f32 (replicated), W: [16384, 4096] f32, scale: [16384] f32
  Each core owns OUT_F/8 = 2048 output features (column parallel).

v6: steady-state PE runs ONLY the main matmuls (N=512, f16 stationary x
fp8 moving, ~216ns each); warm-phase tiles transpose on the PE while W
prep streams in.  Measured constraints that shaped this design:
  - DMA XBAR transposes move 253B packets: fine for per-tile x traffic
    (1MB/27.6us, v5-proven) and for W (17MB), but bulk warm traffic
    must not pile onto it.
  - Concurrent XBAR instructions on two HWDGE queues corrupt each other
    (v3/v4): ALL XBARs live on the ACT queue.
  - In-order engine queues: no op may sit behind another chain's
    long-latency wait.  SP carries ONLY y DMAs; DVE carries W sign/fp8
    + y copies; ACT carries XBARs + warm xT PSUM->SBUF copies,
    emission-interleaved so every wait is already satisfied.
Queues:
  - gpsimd: casting DMAs f32->f16, ordered [Wb0kc0, xc0, Wb0kc1, xc1,
    Wb1, xc2, Wb2, xc3, Wb3, xc4..], then steady x tiles.
  - PE: warm-tile transposes (8 groups of 4 into PSUM) + all MMs.
Scale: reference pins scale=ones, so the fast variant bakes sign into
B (+-1 fp8, exact: (w&0x8000)^0x3C00 on f16) and skips scaling;
kernel() host-checks scale and falls back to an exact f32 DVE multiply
variant otherwise.
Warm: (tile, band) pairs run in arrival-estimate (diagonal) order.
"""

import os
import sys

for _p in ("/opt/trn_rl_repo",):
    if _p not in sys.path and os.path.isdir(_p):
        sys.path.append(_p)

import numpy as np
import concourse.bacc as bacc
import concourse.mybir as mybir
from concourse.tile import TileContext
from concourse.masks import make_identity
from concourse.bass_utils import run_bass_kernel_spmd

TOKENS, IN_F, OUT_F, NCORES = 8192, 4096, 16384, 8
O_SH = OUT_F // NCORES  # 2048 out features per core
P = 128
KT = IN_F // P          # 32 k-subtiles
MT = TOKENS // P        # 64 token tiles
NBAND = 4               # 4 output bands of 512
BAND = O_SH // NBAND    # 512
WARM = 6                # warm token tiles (PE-transposed)
LOOK = 2                # steady-state xT lookahead (tiles)
F8SUB = 16              # trailing k-subtiles per band in fp8 DoubleRow

f32, f16, u16 = mybir.dt.float32, mybir.dt.float16, mybir.dt.uint16
f8 = mybir.dt.float8e4
AF = mybir.ActivationFunctionType

_CACHE = {}
last_result = None


def build(apply_scale: bool):
    nc = bacc.Bacc("TRN2", target_bir_lowering=False, debug=False)
    x = nc.dram_tensor("x", [TOKENS, IN_F], f32, kind="ExternalInput").ap()
    w = nc.dram_tensor("weight", [O_SH, IN_F], f32, kind="ExternalInput").ap()
    scale = nc.dram_tensor("scale", [O_SH], f32, kind="ExternalInput").ap()
    y = nc.dram_tensor("y", [TOKENS, O_SH], f32, kind="ExternalOutput").ap()

    warm = WARM if not apply_scale else WARM - 2

    with TileContext(nc) as tc:
        with (
            tc.tile_pool(name="const", bufs=1) as cpool,
            tc.tile_pool(name="bres", bufs=1) as bpool,
            tc.tile_pool(name="wf16", bufs=2) as wpool,
            tc.tile_pool(name="wtp", bufs=2) as wtpool,
            tc.tile_pool(name="xstage", bufs=3) as xpool,
            tc.tile_pool(name="xtp", bufs=warm + LOOK) as xtpool,
            tc.tile_pool(name="ystage", bufs=2) as ypool,
            tc.tile_pool(name="x8p", bufs=warm + LOOK) as x8pool,
            tc.tile_pool(name="mmps", bufs=6, space="PSUM") as mmps,
            tc.tile_pool(name="tpps", bufs=2, space="PSUM") as tpps,
        ):
            ident = cpool.tile([P, P], f16, tag="ident")
            make_identity(nc, ident)

            scale_bc = None
            if apply_scale:
                scale_p0 = cpool.tile([1, O_SH], f32, tag="scale_p0")
                nc.sync.dma_start(
                    scale_p0[:], scale.rearrange("(a o) -> a o", a=1)
                )
                scale_bc = cpool.tile([P, O_SH], f32, tag="scale_bc")
                nc.gpsimd.partition_broadcast(scale_bc[:], scale_p0[:])

            B = bpool.tile([P, KT, O_SH], f8, tag="B")

            def w_load(ot, kc, kcw):
                wsg16 = wpool.tile([P, kcw], f16, tag="wsg16")
                nc.gpsimd.dma_start(
                    wsg16[:], w[ot * P : (ot + 1) * P, kc * kcw : (kc + 1) * kcw]
                )
                return wsg16

            def w_finish(ot, kc, kcw, wsg16):
                ksub = kcw // P
                nc.vector.tensor_scalar(
                    wsg16[:].bitcast(u16),
                    wsg16[:].bitcast(u16),
                    0x8000,
                    0x3C00,
                    mybir.AluOpType.bitwise_and,
                    mybir.AluOpType.bitwise_xor,
                )
                wT = wtpool.tile([P, ksub, P], f16, tag="wT")
                nc.scalar.dma_start_transpose(wT[:], wsg16[:])
                nc.vector.tensor_copy(
                    B[:, kc * ksub : (kc + 1) * ksub, ot * P : (ot + 1) * P],
                    wT[:],
                )

            def prep_w_chunk(ot, kc, kcw):
                w_finish(ot, kc, kcw, w_load(ot, kc, kcw))

            def load_x(mt):
                xc = xpool.tile([P, IN_F], f16, tag="xc")
                nc.gpsimd.dma_start(xc[:], x[mt * P : (mt + 1) * P, :])
                return xc

            def make_x8(xT):
                x8 = x8pool.tile([P, F8SUB, P], f8, tag="x8")
                nc.vector.tensor_copy(x8[:], xT[:, KT - F8SUB : KT, :])
                return x8

            def transpose_x_pe(xc):
                """warm path: PE transposes + ACT PSUM->SBUF copies."""
                xT = xtpool.tile([P, KT, P], f16, tag="xT")
                for g in range(KT // 4):
                    tp = tpps.tile([P, 512], f16, tag="tp")
                    for j in range(4):
                        ki = g * 4 + j
                        nc.tensor.transpose(
                            tp[:, j * P : (j + 1) * P],
                            xc[:, ki * P : (ki + 1) * P],
                            ident[:],
                        )
                    nc.vector.tensor_copy(
                        xT[:, g * 4 : g * 4 + 4, :],
                        tp[:].rearrange("p (a b) -> p a b", a=4),
                    )
                return xT, make_x8(xT)

            def make_xT_xbar(mt):
                """steady path: casting load + ACT XBAR transpose."""
                xc = xpool.tile([P, IN_F], f16, tag="xc")
                nc.gpsimd.dma_start(xc[:], x[mt * P : (mt + 1) * P, :])
                xT = xtpool.tile([P, KT, P], f16, tag="xT")
                nc.scalar.dma_start_transpose(xT[:], xc[:])
                return xT, make_x8(xT)

            def emit_y(ps, mt, band):
                n0 = band * BAND
                yq = ypool.tile([P, BAND], f32, tag="yq")
                if apply_scale:
                    nc.vector.tensor_tensor(
                        yq[:], ps[:], scale_bc[:, n0 : n0 + BAND],
                        mybir.AluOpType.mult,
                    )
                else:
                    nc.vector.tensor_copy(yq[:], ps[:])
                nc.sync.dma_start(y[mt * P : (mt + 1) * P, n0 : n0 + BAND], yq[:])

            def mm_fp16_part(ps, xT, band):
                n0 = band * BAND
                for k in range(KT - F8SUB):
                    nc.tensor.matmul(
                        ps[:],
                        xT[:, k, :],
                        B[:, k, n0 : n0 + BAND],
                        start=(k == 0),
                        stop=False,
                    )

            def mm_dr_part(ps, x8, band):
                n0 = band * BAND
                for j in range(F8SUB // 2):
                    k0 = KT - F8SUB + 2 * j
                    nc.tensor.matmul(
                        ps[:],
                        x8[:, 2 * j : 2 * j + 2, :],
                        B[:, k0 : k0 + 2, n0 : n0 + BAND],
                        start=False,
                        stop=(j == F8SUB // 2 - 1),
                        perf_mode=mybir.MatmulPerfMode.DoubleRow,
                    )

            def mm_band(mt, band, xp):
                xT, x8 = xp
                ps = mmps.tile([P, BAND], f32, tag="ps")
                mm_fp16_part(ps, xT, band)
                mm_dr_part(ps, x8, band)
                emit_y(ps, mt, band)

            # --- warm phase ---
            # gpsimd load order: Wb0kc0, xc0, Wb0kc1, xc1, Wb1, xc2,
            # Wb2(deferred), xc3, Wb3(deferred), xc4..; band2/3 prep is
            # emitted a few pairs into the MM sweep so no queue blocks.
            xcs = {}
            for oi in range(4):
                prep_w_chunk(oi, 0, 2048)
            xcs[0] = load_x(0)
            for oi in range(4):
                prep_w_chunk(oi, 1, 2048)
            xcs[1] = load_x(1)
            for oi in range(4):
                prep_w_chunk(4 + oi, 0, IN_F)   # band 1, full rows
            xcs[2] = load_x(2)

            pairs = sorted(
                ((t, b) for t in range(warm) for b in range(NBAND)),
                key=lambda p: (max(36 + 12.0 * p[0], 30 + 22.0 * p[1]), p[0]),
            )
            xts = {}
            for i, (t, b) in enumerate(pairs):
                if i == 2:
                    for oi in range(4):
                        prep_w_chunk(8 + oi, 0, IN_F)   # band 2
                    xcs[3] = load_x(3)
                if i == 6:
                    for oi in range(4):
                        prep_w_chunk(12 + oi, 0, IN_F)  # band 3
                if i == 8:
                    for mt in range(4, warm):
                        xcs[mt] = load_x(mt)
                if t not in xts:
                    xts[t] = transpose_x_pe(xcs[t])
                mm_band(t, b, xts[t])

            # --- steady phase with xT lookahead ---
            for mt in range(warm, MT + LOOK):
                if mt < MT:
                    xts[mt] = make_xT_xbar(mt)
                rt = mt - LOOK
                if rt >= warm:
                    xT, x8 = xts.pop(rt)
                    pss = [
                        mmps.tile([P, BAND], f32, name=f"sps{rt}_{b}", tag="ps")
                        for b in range(NBAND)
                    ]
                    for band in range(NBAND):
                        mm_fp16_part(pss[band], xT, band)
                    for band in range(NBAND):
                        mm_dr_part(pss[band], x8, band)
                        emit_y(pss[band], rt, band)

    nc.finalize()
    return nc


def _get_nc(apply_scale: bool):
    key = "scale" if apply_scale else "ones"
    if key not in _CACHE:
        _CACHE[key] = build(apply_scale)
    return _CACHE[key]


def kernel(x, weight, scale):
    global last_result
    x = np.ascontiguousarray(np.asarray(x, dtype=np.float32))
    weight = np.ascontiguousarray(np.asarray(weight, dtype=np.float32))
    scale = np.ascontiguousarray(np.asarray(scale, dtype=np.float32))
    apply_scale = not bool(np.all(scale == 1.0))
    nc = _get_nc(apply_scale)
    in_maps = [
        {
            "x": x,
            "weight": np.ascontiguousarray(weight[c * O_SH : (c + 1) * O_SH]),
            "scale": np.ascontiguousarray(scale[c * O_SH : (c + 1) * O_SH]),
        }
        for c in range(NCORES)
    ]
    res = run_bass_kernel_spmd(nc, in_maps, list(range(NCORES)))
    last_result = res
    return np.concatenate([res.results[c]["y"] for c in range(NCORES)], axis=1)


if __name__ == "__main__":
    rng = np.random.default_rng(0)
    xv = rng.standard_normal((TOKENS, IN_F), dtype=np.float32)
    wv = rng.standard_normal((OUT_F, IN_F), dtype=np.float32)
    sv = np.ones(OUT_F, dtype=np.float32)
    yv = kernel(xv, wv, sv)
    print("out shape:", yv.shape, yv.dtype)



# revision 6
# speedup vs baseline: 1.1022x; 1.1022x over previous
"""BitLinear kernel for Trainium2, 8 NeuronCores, column-parallel.

y[t, o] = sum_i x[t, i] * sign(W[o, i]) * scale[o]
  x: [8192, 4096] f32 (replicated), W: [16384, 4096] f32, scale: [16384] f32
  Each core owns OUT_F/8 = 2048 output features (column parallel).

v7: PE floor is 96 matmuls/tile x 216ns = 20.7us/tile (16 f16 + 8 fp8
DoubleRow subtile-pairs per 512-out band; the 16/16 f16-fp8 split is
error-optimal at rel_err 1.87e-2 vs the 1.95e-2 gate).  v6 measured
1.731ms vs the 1.327ms floor; the overhead was (a) ~190us warm-phase PE
idle while W streamed through the packet-rate-limited XBAR (q10 caps at
~200M 254B-packets/s), (b) ~140us steady stalls waiting on the x
transpose chain + p-state re-ramps after each gap.  v7 changes:
  - W never touches the XBAR.  Half the chunks cast-DMA f32->f16 on the
    gpsimd queue (casting DMAs are gpsimd-only) + DVE sign bit-trick
    ((w&0x8000)^0x3C00); the other half load raw f32 on the SP queue +
    ACT Sign-activation f32->f16.  All chunks then PE-transpose
    (55-107ns/block) into PSUM f16 and copy-cast PSUM->B fp8 (split
    DVE/ACT).  All 4 bands land by ~180us.
  - x transposes split into two half-tile XBARs (subtiles 0-15 f16 for
    the f16 matmuls, 16-31 feeding the DVE fp8 cast) for finer
    dependency granularity; LOOK=3 tiles of lookahead.
  - y is emitted to SBUF as f16 (adds ~5e-4 rel err in quadrature --
    negligible) and stored as f16; kernel() casts back to f32 on host.
    Steady tiles store one [128, 2048] DMA (4KB rows); warm tiles store
    per-band to bound staging while SP is still streaming W.
  - warm emission order comes from a small static event simulation so
    no in-order engine queue blocks on a not-yet-ready dependency.
Engines: gpsimd=x casts + W casts, SP=W raw loads + y stores, ACT=XBARs
+ W sign + some B copies, DVE=W bitops + x fp8 casts + y emits + some B
copies, PE=W transposes + matmuls.
Scale: reference pins scale=ones, so the fast variant bakes sign into
B (+-1 fp8 exact) and skips scaling; kernel() host-checks scale and
falls back to a scaled-multiply DVE variant otherwise.
"""

import os
import sys

for _p in ("/opt/trn_rl_repo",):
    if _p not in sys.path and os.path.isdir(_p):
        sys.path.append(_p)

import numpy as np
import concourse.bacc as bacc
import concourse.mybir as mybir
from concourse.tile import TileContext
from concourse.masks import make_identity
from concourse.bass_utils import run_bass_kernel_spmd

TOKENS, IN_F, OUT_F, NCORES = 8192, 4096, 16384, 8
O_SH = OUT_F // NCORES  # 2048 out features per core
P = 128
KT = IN_F // P          # 32 k-subtiles
MT = TOKENS // P        # 64 token tiles
NBAND = 4               # 4 output bands of 512
BAND = O_SH // NBAND    # 512
F8SUB = 16              # trailing k-subtiles per band in fp8 DoubleRow
WARM = 8                # tiles emitted by the warm scheduler
LOOK = 3                # steady-state lookahead (tiles)
NCHUNK = 32             # W chunks: (o-block 0..15) x (k-half 0..1)

f32, f16, u16 = mybir.dt.float32, mybir.dt.float16, mybir.dt.uint16
f8 = mybir.dt.float8e4
AF = mybir.ActivationFunctionType

_CACHE = {}
last_result = None


def _chunk_on_sp(c):
    # o-blocks 2,3 of each band load raw f32 on SP; 0,1 cast on gpsimd
    return (c >> 1) % 4 >= 2


def build(apply_scale: bool):
    nc = bacc.Bacc("TRN2", target_bir_lowering=False, debug=False)
    x = nc.dram_tensor("x", [TOKENS, IN_F], f32, kind="ExternalInput").ap()
    w = nc.dram_tensor("weight", [O_SH, IN_F], f32, kind="ExternalInput").ap()
    scale = nc.dram_tensor("scale", [O_SH], f32, kind="ExternalInput").ap()
    y = nc.dram_tensor("y", [TOKENS, O_SH], f16, kind="ExternalOutput").ap()

    xta_bufs = 9 if not apply_scale else 7
    yq_bufs = 16 if not apply_scale else 9

    with TileContext(nc) as tc:
        with (
            tc.tile_pool(name="const", bufs=1) as cpool,
            tc.tile_pool(name="bres", bufs=1) as bpool,
            tc.tile_pool(name="wq0", bufs=2) as wq0pool,
            tc.tile_pool(name="wsp32", bufs=2) as wsp32pool,
            tc.tile_pool(name="wsp16", bufs=2) as wsp16pool,
            tc.tile_pool(name="xstage", bufs=2) as xpool,
            tc.tile_pool(name="xta", bufs=xta_bufs) as xtapool,
            tc.tile_pool(name="xtb", bufs=3) as xtbpool,
            tc.tile_pool(name="x8p", bufs=11) as x8pool,
            tc.tile_pool(name="yst", bufs=2) as ypool,
            tc.tile_pool(name="yq", bufs=yq_bufs) as yqpool,
            tc.tile_pool(name="mmps", bufs=6, space="PSUM") as mmps,
            tc.tile_pool(name="tpps", bufs=2, space="PSUM") as tpps,
        ):
            ident = cpool.tile([P, P], f16, tag="ident")
            make_identity(nc, ident)

            scale_bc = None
            if apply_scale:
                scale_p0 = cpool.tile([1, O_SH], f32, tag="scale_p0")
                nc.sync.dma_start(
                    scale_p0[:], scale.rearrange("(a o) -> a o", a=1)
                )
                scale_bc = cpool.tile([P, O_SH], f32, tag="scale_bc")
                nc.gpsimd.partition_broadcast(scale_bc[:], scale_p0[:])

            B = bpool.tile([P, KT, O_SH], f8, tag="B")

            # ---------- op emitters (called in simulated order) ----------
            wsg = {}      # chunk -> sign-converted f16 tile
            wraw = {}     # SP chunk -> raw f32 tile
            tpt = {}      # (chunk, bank) -> PSUM f16 transpose tile
            xcs, xtas, xtbs, x8s = {}, {}, {}, {}
            psb = {}      # (t, band) -> PSUM accumulation tile
            ystw = {}     # warm (t, band) -> yq tile

            def e_wcast(c):
                ot, kc = c >> 1, c & 1
                sl = w[ot * P : (ot + 1) * P, kc * 2048 : (kc + 1) * 2048]
                if _chunk_on_sp(c):
                    t = wsp32pool.tile([P, 2048], f32, tag="wraw")
                    nc.sync.dma_start(t[:], sl)
                    wraw[c] = t
                else:
                    t = wq0pool.tile([P, 2048], f16, tag="wsg")
                    nc.gpsimd.dma_start(t[:], sl)
                    wsg[c] = t

            def e_conv(c):
                # sign -> +-1 f16
                if _chunk_on_sp(c):
                    t = wsp16pool.tile([P, 2048], f16, tag="wsg16")
                    nc.scalar.activation(t[:], wraw.pop(c)[:], AF.Sign)
                    wsg[c] = t
                else:
                    t = wsg[c]
                    nc.vector.tensor_scalar(
                        t[:].bitcast(u16),
                        t[:].bitcast(u16),
                        0x8000,
                        0x3C00,
                        mybir.AluOpType.bitwise_and,
                        mybir.AluOpType.bitwise_xor,
                    )

            def e_tbank(c, g):
                # transpose 8 k-subtiles of chunk c into one PSUM bank
                t = wsg[c]
                tp = tpps.tile([P, 1024], f16, tag="tp")
                for j in range(8):
                    nc.tensor.transpose(
                        tp[:, j * P : (j + 1) * P],
                        t[:, (g * 8 + j) * P : (g * 8 + j + 1) * P],
                        ident[:],
                    )
                tpt[(c, g)] = tp

            def e_bcopy(c, g):
                ot, kc = c >> 1, c & 1
                k0 = kc * 16 + g * 8
                dst = B[:, k0 : k0 + 8, ot * P : (ot + 1) * P]
                src = tpt.pop((c, g))[:].rearrange("p (a b) -> p a b", a=8)
                if _chunk_on_sp(c):
                    nc.scalar.activation(dst, src, AF.Copy)
                else:
                    nc.vector.tensor_copy(dst, src)

            def e_xcast(t):
                xc = xpool.tile([P, IN_F], f16, tag="xc")
                nc.gpsimd.dma_start(xc[:], x[t * P : (t + 1) * P, :])
                xcs[t] = xc

            def e_xbarA(t):
                xta = xtapool.tile([P, KT - F8SUB, P], f16, tag="xta")
                nc.scalar.dma_start_transpose(xta[:], xcs[t][:, 0:2048])
                xtas[t] = xta

            def e_xbarB(t):
                xtb = xtbpool.tile([P, F8SUB, P], f16, tag="xtb")
                nc.scalar.dma_start_transpose(xtb[:], xcs[t][:, 2048:IN_F])
                xtbs[t] = xtb

            def e_x8(t):
                x8 = x8pool.tile([P, F8SUB, P], f8, tag="x8")
                nc.vector.tensor_copy(x8[:], xtbs.pop(t)[:])
                x8s[t] = x8

            def e_mm(t, b):
                n0 = b * BAND
                ps = mmps.tile([P, BAND], f32, tag="ps")
                xta, x8 = xtas[t], x8s[t]
                for k in range(KT - F8SUB):
                    nc.tensor.matmul(
                        ps[:],
                        xta[:, k, :],
                        B[:, k, n0 : n0 + BAND],
                        start=(k == 0),
                        stop=False,
                    )
                for j in range(F8SUB // 2):
                    k0 = KT - F8SUB + 2 * j
                    nc.tensor.matmul(
                        ps[:],
                        x8[:, 2 * j : 2 * j + 2, :],
                        B[:, k0 : k0 + 2, n0 : n0 + BAND],
                        start=False,
                        stop=(j == F8SUB // 2 - 1),
                        perf_mode=mybir.MatmulPerfMode.DoubleRow,
                    )
                psb[(t, b)] = ps

            def e_yemit(t, b, dst, n0_dst):
                ps = psb.pop((t, b))
                if apply_scale:
                    nc.vector.tensor_tensor(
                        dst[:, n0_dst : n0_dst + BAND],
                        ps[:],
                        scale_bc[:, b * BAND : (b + 1) * BAND],
                        mybir.AluOpType.mult,
                    )
                else:
                    nc.vector.tensor_copy(dst[:, n0_dst : n0_dst + BAND], ps[:])

            # ---------- warm phase: static event simulation ----------
            # costs (us, rough): tuned from v6 trace
            TQ0W, TQ0X, TSPW = 7.0, 9.5, 9.5
            TXB, TMM, TTB = 6.5, 5.2, 0.55
            TBC, TX8, TCV_DVE, TCV_ACT, TYE, TYD = 0.9, 1.6, 0.6, 1.8, 0.5, 1.4

            # queue plans: W chunk c = (o_block<<1)|k_half; band b owns
            # o_blocks 4b..4b+3; per band, blocks 0,1 -> gpsimd cast,
            # blocks 2,3 -> SP raw f32.
            q0w, spw = [], []
            for b in range(NBAND):
                for kc in range(2):
                    q0w += [((4 * b) << 1) | kc, ((4 * b + 1) << 1) | kc]
                    spw += [((4 * b + 2) << 1) | kc, ((4 * b + 3) << 1) | kc]
            # interleave warm x casts into q0 at a steady cadence
            q0 = []
            xi = 0
            for i, c in enumerate(q0w):
                if i % 2 == 0 and xi < WARM:
                    q0.append(("x", xi))
                    xi += 1
                q0.append(("W", c))
            while xi < WARM:
                q0.append(("x", xi))
                xi += 1

            ends = {}
            clk = 0.0
            for kind, idx in q0:
                clk += TQ0X if kind == "x" else TQ0W
                ends[(kind, idx)] = clk
            spclk = 0.0
            for c in spw:
                spclk += TSPW
                ends[("W", c)] = spclk

            # event sim for DVE/ACT/PE/SP-y emission order
            pend = []
            for c in range(NCHUNK):
                if _chunk_on_sp(c):
                    pend.append((("cv", c), "act", TCV_ACT, [("W", c)], 1))
                    bce = "act"
                else:
                    pend.append((("cv", c), "dve", TCV_DVE, [("W", c)], 1))
                    bce = "dve"
                for g in range(2):
                    pend.append((("tb", c, g), "pe", TTB, [("cv", c)], 0))
                    pend.append((("bc", c, g), bce, TBC, [("tb", c, g)], 2))
            for t in range(WARM):
                pend.append((("xa", t), "act", TXB, [("x", t)], 0))
                pend.append((("xb", t), "act", TXB, [("xa", t)], 0))
                pend.append((("x8", t), "dve", TX8, [("xb", t)], 0))
            bdep = {}
            for b in range(NBAND):
                cs = [((4 * b + i) << 1) | kc for i in range(4) for kc in range(2)]
                bdep[b] = [("bc", c, g) for c in cs for g in range(2)]
            for t in range(WARM):
                for b in range(NBAND):
                    pend.append(
                        (("mm", t, b), "pe", TMM, [("xa", t), ("x8", t)] + bdep[b], 1)
                    )
                    pend.append((("ye", t, b), "dve", TYE, [("mm", t, b)], 2))
                    pend.append((("yd", t, b), "sp", TYD, [("ye", t, b)], 3))

            engclk = {"dve": 0.0, "act": 0.0, "pe": 0.0, "sp": spclk}
            emitted = []
            pend_d = {p[0]: p for p in pend}
            while pend_d:
                best, best_key = None, None
                for key, (k, eng, cost, deps, pri) in pend_d.items():
                    if any(d not in ends for d in deps):
                        continue
                    start = max(engclk[eng], max(ends[d] for d in deps))
                    o = (start, pri)
                    if best is None or o < best:
                        best, best_key = o, (k, eng, cost, deps, pri)
                k, eng, cost, deps, pri = best_key
                start = max(engclk[eng], max(ends[d] for d in deps))
                ends[k] = start + cost
                engclk[eng] = start + cost
                del pend_d[k]
                emitted.append(k)

            # emit DMAs first (their per-queue order is the plan order;
            # cross-engine program order doesn't matter for scheduling)
            for kind, idx in q0:
                if kind == "x":
                    e_xcast(idx)
                else:
                    e_wcast(idx)
            for c in spw:
                e_wcast(c)

            # emit DVE/ACT/PE/SP warm ops in simulated order
            for k in emitted:
                if k[0] == "cv":
                    e_conv(k[1])
                elif k[0] == "tb":
                    e_tbank(k[1], k[2])
                elif k[0] == "bc":
                    e_bcopy(k[1], k[2])
                elif k[0] == "xa":
                    e_xbarA(k[1])
                elif k[0] == "xb":
                    e_xbarB(k[1])
                elif k[0] == "x8":
                    e_x8(k[1])
                elif k[0] == "mm":
                    e_mm(k[1], k[2])
                elif k[0] == "ye":
                    t, b = k[1], k[2]
                    yq = yqpool.tile([P, BAND], f16, tag="yq")
                    ystw[(t, b)] = yq
                    e_yemit(t, b, yq, 0)
                elif k[0] == "yd":
                    t, b = k[1], k[2]
                    nc.sync.dma_start(
                        y[t * P : (t + 1) * P, b * BAND : (b + 1) * BAND],
                        ystw.pop((t, b))[:],
                    )

            # ---------- steady phase ----------
            for mt in range(WARM, MT + LOOK):
                if mt < MT:
                    e_xcast(mt)
                    e_xbarA(mt)
                    e_xbarB(mt)
                    e_x8(mt)
                rt = mt - LOOK
                if rt >= WARM:
                    yst = ypool.tile([P, O_SH], f16, tag="yst")
                    for b in range(NBAND):
                        e_mm(rt, b)
                        e_yemit(rt, b, yst, b * BAND)
                    nc.sync.dma_start(y[rt * P : (rt + 1) * P, :], yst[:])

    nc.finalize()
    return nc


def _get_nc(apply_scale: bool):
    key = "scale" if apply_scale else "ones"
    if key not in _CACHE:
        _CACHE[key] = build(apply_scale)
    return _CACHE[key]


def kernel(x, weight, scale):
    global last_result
    x = np.ascontiguousarray(np.asarray(x, dtype=np.float32))
    weight = np.ascontiguousarray(np.asarray(weight, dtype=np.float32))
    scale = np.ascontiguousarray(np.asarray(scale, dtype=np.float32))
    apply_scale = not bool(np.all(scale == 1.0))
    nc = _get_nc(apply_scale)
    in_maps = [
        {
            "x": x,
            "weight": np.ascontiguousarray(weight[c * O_SH : (c + 1) * O_SH]),
            "scale": np.ascontiguousarray(scale[c * O_SH : (c + 1) * O_SH]),
        }
        for c in range(NCORES)
    ]
    res = run_bass_kernel_spmd(nc, in_maps, list(range(NCORES)))
    last_result = res
    return np.concatenate(
        [np.asarray(res.results[c]["y"]).astype(np.float32) for c in range(NCORES)],
        axis=1,
    )


if __name__ == "__main__":
    rng = np.random.default_rng(0)
    xv = rng.standard_normal((TOKENS, IN_F), dtype=np.float32)
    wv = rng.standard_normal((OUT_F, IN_F), dtype=np.float32)
    sv = np.ones(OUT_F, dtype=np.float32)
    yv = kernel(xv, wv, sv)
    print("out shape:", yv.shape, yv.dtype)
